# revision 1
# baseline (speedup 1.0000x reference)
"""Dice-loss (segment_reduce) kernel for 8 Trainium2 NeuronCores.

Full inputs: input (4,5,128,128,128) f32, target (4,128,128,128) int64.
Output: scalar mean dice, shape (1,), f32 — matches the jax reference.

Sharding: 8 cores = 4 batches x 2 spatial halves. Each core computes, for
its 1,048,576 positions, per-class counts for classes 1..4:
  P_c = #(x_c == max over classes)        (argmax one-hot; exact ties
                                           overcount, prob ~1e-6 effect)
  I_c = #((x_c == max) and target == c)
Target-class counts T_c are exact and cheap on the host (np.bincount).
The tiny per-core count vectors are gathered to the host, which forms
dice = (2I+eps)/(P+T+eps) and the final mean.

Per core the device streams 21 MiB (x: 20 MiB f32, target: 1 MiB int8).
All compare work is VectorE (the only 2-source-capable engine in this
toolchain): per chunk of 2048 positions x 128 partitions:
  4x tensor_tensor max   (tree max over 5 classes)
  4x scalar_tensor_tensor eq_c = (x_c >= M), per-partition count fused
  4x scalar_tensor_tensor (t == c) * eq_c, per-partition count fused
"""

import sys

sys.path.insert(0, "/opt/trn_rl_repo")

import numpy as np
import concourse.bass as bass
import concourse.mybir as mybir
from concourse.tile import TileContext
from concourse.bass_utils import run_bass_kernel_spmd

F32 = mybir.dt.float32
BF16 = mybir.dt.bfloat16
I8 = mybir.dt.int8
Alu = mybir.AluOpType
Act = mybir.ActivationFunctionType

B, C = 4, 5
N = 128 * 128 * 128          # spatial positions per batch
NCORES = 8
HALF = N // 2                # positions per core
P = 128                      # SBUF partitions
# Ramped chunk sizes (free-dim elems per partition, sum = HALF/P = 8192):
# small first chunks shorten the DMA pipeline-fill stall, small last chunk
# shortens the tail before the accumulator writeback.
CHUNKS = (256, 256, 512, 1024, 2048, 2048, 2048)
NCH = len(CHUNKS)
assert sum(CHUNKS) == HALF // P
EPS = 1e-5

_prog_cache = {}


def _legalize_waits(nc):
    """Split multi-wait instructions: this walrus build's codegen allows only
    one embedded sync-wait per instruction ("Too many sync wait commands").
    Move extra waits onto standalone EventSemaphore instructions inserted
    just before, on the same engine queue — semantically identical."""
    n_new = 0
    for bb in nc.main_func.blocks:
        insts = list(bb.instructions)
        out = []
        changed = False
        for ins in insts:
            si = ins.sync_info
            waits = list(si.on_wait) if si and si.on_wait else []
            if len(waits) > 1:
                for w in waits[:-1]:
                    ev = mybir.InstEventSemaphore(
                        name=f"legalw-{n_new}", ins=[], outs=[]
                    )
                    n_new += 1
                    ev.engine = ins.engine
                    ev.sync_info = mybir.SyncInfo(on_wait=[w], on_update=[])
                    nc.register_instruction(ev)
                    out.append(ev)
                ins.sync_info = mybir.SyncInfo(
                    on_wait=[waits[-1]], on_update=list(si.on_update or [])
                )
                changed = True
            out.append(ins)
        if changed:
            live = bb.instructions
            live.clear()
            live.extend(out)
    return n_new


def _build_program():
    nc = bass.Bass()

    x = nc.dram_tensor("x", [C, HALF], F32, kind="ExternalInput")
    t = nc.dram_tensor("t", [HALF], I8, kind="ExternalInput")
    yp = nc.dram_tensor("yp", [P, 4 * NCH], F32, kind="ExternalOutput")
    yi = nc.dram_tensor("yi", [P, 4 * NCH], F32, kind="ExternalOutput")

    # x viewed as (C, P, 8192): partition p owns elements [p*8192,(p+1)*8192)
    # of each class block; chunk ch covers free-dim cols [off, off+m).
    xr = x[:].rearrange("c (p f) -> p c f", p=P)
    tr = t[:].rearrange("(p f) -> p f", p=P)

    with TileContext(nc) as tc:
        with (
            tc.tile_pool(name="xin", bufs=3) as pool_x,
            tc.tile_pool(name="tin", bufs=3) as pool_t,
            tc.tile_pool(name="work", bufs=1) as pool_w,
            tc.tile_pool(name="accs", bufs=1) as pool_a,
        ):
            accP = pool_a.tile([P, 4 * NCH], F32)
            accI = pool_a.tile([P, 4 * NCH], F32)

            off = 0
            for ch, M in enumerate(CHUNKS):
                xt = pool_x.tile([P, C, M], F32, tag="xt")
                tt = pool_t.tile([P, M], I8, tag="tt")
                # split the class load across two DMA queues: more aggregate
                # bandwidth during ramp-up, and the max tree's first operands
                # (classes 0-1) arrive without waiting for the whole chunk
                nc.sync.dma_start(out=xt[:, 0:2, :], in_=xr[:, 0:2, off : off + M])
                nc.sync.dma_start(out=xt[:, 2:5, :], in_=xr[:, 2:5, off : off + M])
                nc.sync.dma_start(out=tt[:], in_=tr[:, off : off + M])
                off += M

                # VectorE: max over the 5 classes (tree).  All consumed
                # same-engine within the chunk, so bufs=1 tiles suffice.
                ma = pool_w.tile([P, M], F32, tag="ma")
                mb = pool_w.tile([P, M], F32, tag="mb")
                mc_ = pool_w.tile([P, M], F32, tag="mc")
                mx = pool_w.tile([P, M], F32, tag="mx")
                nc.vector.tensor_tensor(out=ma[:], in0=xt[:, 0, :], in1=xt[:, 1, :], op=Alu.max)
                nc.vector.tensor_tensor(out=mb[:], in0=xt[:, 2, :], in1=xt[:, 3, :], op=Alu.max)
                nc.vector.tensor_tensor(out=mc_[:], in0=ma[:], in1=mb[:], op=Alu.max)
                nc.vector.tensor_tensor(out=mx[:], in0=mc_[:], in1=xt[:, 4, :], op=Alu.max)

                # Per class: eq_c = (x_c >= M) then inter_c = (t==c)*eq_c,
                # both with fused per-partition counts. Interleaved so the
                # accumulator readouts spread across the chunk.
                junk = pool_w.tile([P, M], BF16, tag="junk")
                for c in range(1, C):
                    eq = pool_w.tile([P, M], BF16, tag=f"eq{c}", name=f"eq{c}_{ch}")
                    col = ch * 4 + c - 1
                    nc.vector.scalar_tensor_tensor(
                        out=eq[:], in0=xt[:, c, :], scalar=0.0, in1=mx[:],
                        op0=Alu.add, op1=Alu.is_ge,
                        accum_out=accP[:, col : col + 1],
                    )
                    nc.vector.scalar_tensor_tensor(
                        out=junk[:], in0=tt[:], scalar=float(c), in1=eq[:],
                        op0=Alu.is_equal, op1=Alu.mult,
                        accum_out=accI[:, col : col + 1],
                    )

            nc.sync.dma_start(out=yp[:], in_=accP[:])
            nc.sync.dma_start(out=yi[:], in_=accI[:])

    _legalize_waits(nc)
    return nc


def _get_program():
    if "nc" not in _prog_cache:
        _prog_cache["nc"] = _build_program()
    return _prog_cache["nc"]


def _run(input, target, trace=False, trace_kwargs=None):
    inp = np.asarray(input)
    tgt = np.asarray(target)
    assert inp.shape == (B, C, 128, 128, 128), inp.shape
    assert tgt.shape == (B, 128, 128, 128), tgt.shape

    inp_r = inp.reshape(B, C, N)
    tgt_r = tgt.reshape(B, N)

    in_maps = []
    t8s = []
    for core in range(NCORES):
        b, h = core // 2, core % 2
        xs = np.ascontiguousarray(inp_r[b, :, h * HALF : (h + 1) * HALF])
        ts_ = tgt_r[b, h * HALF : (h + 1) * HALF].astype(np.int8)
        t8s.append(ts_)
        in_maps.append({"x": xs, "t": ts_})

    nc = _get_program()
    kw = {}
    if trace:
        kw["trace"] = True
        if trace_kwargs:
            kw.update(trace_kwargs)
    res = run_bass_kernel_spmd(nc, in_maps, list(range(NCORES)), **kw)

    # host combine: per (batch, class) counts from the two half-cores
    Pc = np.zeros((B, C), np.float64)
    Tc = np.zeros((B, C), np.float64)
    Ic = np.zeros((B, C), np.float64)
    for core in range(NCORES):
        b = core // 2
        r = res.results[core]
        Tc[b] += np.bincount(t8s[core], minlength=C)
        for c in range(1, C):
            cols = slice(c - 1, 4 * NCH, 4)
            Pc[b, c] += r["yp"][:, cols].sum()
            Ic[b, c] += r["yi"][:, cols].sum()

    inter = Ic[:, 1:].astype(np.float32)
    union = (Pc[:, 1:] + Tc[:, 1:]).astype(np.float32)
    dice = (2.0 * inter + np.float32(EPS)) / (union + np.float32(EPS))
    out = np.array([dice.mean(dtype=np.float32)], dtype=np.float32)
    return out, res


def kernel(input, target):
    out, _ = _run(input, target, trace=False)
    return out

